# revision 26
# baseline (speedup 1.0000x reference)
"""Trainium2 Bass kernel for nn_MixedGatedMLP (4-bit quantized gated MLP + LoRA).

Strategy v3: tensor-parallel over d_ff across 8 NeuronCores (F padded
11008->11264, FS=1408 rows/core).  Host ships unpacked nibble planes (bf16) and
expanded per-element absmax planes; the device does the codebook lookup.

Dequant v3: 15 tensor_scalar ops with IMMEDIATE code scalars (4x DVE mode,
~693ns per [128,2048] bf16 group) using the identity
  c[X] = c0 + sum_{k=1..15} (c_k - c0) * 1[X==k],
accumulated via scalar_tensor_tensor fused adds (also 4x); a few terms are
summed on GPSIMD to balance engines.  The final (acc + c0) * S scale is one
STT.  The down-proj LoRA (a_d @ b_d) is computed on the PE and folded into the
dequantized down weights, so phase 2 is pure matmul.

Phase 1 streams x in 512-token tiles over uniform 2-fgroup slices of the
gate/up weights; dequant of slice s+1 (plus a batch of down-weight pieces)
overlaps the matmuls of slice s via an issue-order task queue.  The gate/up
LoRA projection x @ [a_g | a_u] is fused into one 64-wide stationary matmul.
SiLU runs on ACT, the gating multiply on GPSIMD, x3 goes to DRAM bf16.

Phase 2 holds the full dequantized down matrix in SBUF (90KB/partition),
reads x3 once per token group, accumulates y3 in 8 rotating PSUM banks, and
fires a bf16 ReduceScatter per token quarter overlapped with the next
quarter's matmuls.  Core i returns tokens {1024q + 128i ..} per quarter.
"""

import sys

for _p in ("/opt/trn_rl_repo", "/root/.axon_site/_ro/trn_rl_repo"):
    if _p not in sys.path:
        sys.path.append(_p)

from contextlib import ExitStack

import numpy as np
import ml_dtypes

import concourse.bass as bass
import concourse.mybir as mybir
import concourse.tile as tile
from concourse import bacc
from concourse.bass_utils import run_bass_kernel_spmd

BF16 = ml_dtypes.bfloat16
ALU = mybir.AluOpType
AFT = mybir.ActivationFunctionType


class Cfg:
    def __init__(self, D=4096, T=4096, F=11008, R=16, block=64, ncores=8,
                 use_silu=True):
        self.D = D
        self.T = T
        self.F = F
        self.R = R
        self.block = block
        self.ncores = ncores
        self.use_silu = use_silu

        unit = 128 * ncores
        self.FP = ((F + unit - 1) // unit) * unit   # padded d_ff
        self.FS = self.FP // ncores                 # per-core f rows
        self.NFG = self.FS // 128                   # 128-row f groups
        self.DCH = D // 128                         # 128-row d chunks

        # token tiling
        self.TTW = min(512, T)                      # phase-1 token tile width
        self.NT = T // self.TTW
        self.NTG = T // 128                         # phase-2 token groups

        # phase-1 f slices in fgroup units: small first slice for the bubble
        if self.NFG == 11:
            self.slices = [1, 2, 2, 2, 2, 2]
        else:
            self.slices = [1] * self.NFG

        self.deq_w = 2048        # dequant group width (free elems)
        self.pool_terms = 0      # codebook terms accumulated on gpsimd

        # phase 2
        self.n_q = 8 if T >= 4096 else 2            # token RS chunks
        self.TQ = T // self.n_q
        self.TQC = self.TQ // ncores                # rows per core per quarter
        self.TS = T // ncores
        self.NDJ = D // 512                         # 512-wide d chunks

    def slice_groups(self, nfg):
        """Dequant chunk-groups (c0, G) for a slice of nfg fgroups."""
        fw = nfg * 128
        G = max(1, min(self.deq_w // fw, 16, self.DCH))
        while self.DCH % G:
            G -= 1
        return [(c0, G) for c0 in range(0, self.DCH, G)]


def build_graph(cfg: Cfg, code):
    """code: tuple of 16 python floats, baked as immediates."""
    nc = bacc.Bacc(None, num_devices=cfg.ncores)
    dt = mybir.dt
    D, T, FS, R, NFG = cfg.D, cfg.T, cfg.FS, cfg.R, cfg.NFG
    TTW = cfg.TTW
    rg = [list(range(cfg.ncores))]
    c0v = float(code[0])
    cdel = [float(code[k]) - c0v for k in range(16)]

    # const APs for ACT-island activation biases (sundagen needs bias tensors)
    def reg_const(v):
        v = float(v)
        key = (dt.float32, v)
        if key not in nc.const_aps.aps:
            t = nc.alloc_sbuf_tensor(
                f"constx-{len(nc.const_aps.aps)}", [128, 1], dt.float32)
            nc.gpsimd.memset(t.ap(), v)
            nc.const_aps.aps[key] = t.ap()
    for _k in range(1, 16):
        reg_const(-float(_k))
        if abs(cdel[_k]) > 1e-30:
            reg_const(float(np.log(abs(cdel[_k]))))
    nc.all_engine_barrier()

    # ---- external inputs (per-core) ----
    xT = nc.dram_tensor("xT", [D, T], dt.bfloat16, kind="ExternalInput")
    g_nib = nc.dram_tensor("g_nib", [D, FS], dt.bfloat16, kind="ExternalInput")
    u_nib = nc.dram_tensor("u_nib", [D, FS], dt.bfloat16, kind="ExternalInput")
    d_nib = nc.dram_tensor("d_nib", [FS, D], dt.bfloat16, kind="ExternalInput")
    g_amp = nc.dram_tensor("g_amp", [D, FS], dt.bfloat16, kind="ExternalInput")
    u_amp = nc.dram_tensor("u_amp", [D, FS], dt.bfloat16, kind="ExternalInput")
    d_amp = nc.dram_tensor("d_amp", [FS, D], dt.bfloat16, kind="ExternalInput")
    # gate/up lora: a_gu [D, 64] (a_g cols 0:16, a_u cols 32:48);
    # b_gu [64, FS] (b_g rows 0:16, b_u rows 32:48)
    a_gu = nc.dram_tensor("a_gu", [D, 4 * R], dt.bfloat16, kind="ExternalInput")
    b_gu = nc.dram_tensor("b_gu", [4 * R, FS], dt.bfloat16, kind="ExternalInput")
    # down lora: a_dT [R, FS] (= a_d.T), b_d [R, D]
    a_dT = nc.dram_tensor("a_dT", [R, FS], dt.bfloat16, kind="ExternalInput")
    b_d = nc.dram_tensor("b_d", [R, D], dt.bfloat16, kind="ExternalInput")

    y_out = nc.dram_tensor("y_out", [cfg.TS, D], dt.float32, kind="ExternalOutput")

    # ---- internal DRAM ----
    x3_dram = nc.dram_tensor("x3_dram", [FS, T], dt.bfloat16, kind="Internal")
    wd_dram = nc.dram_tensor("wd_dram", [FS, D], dt.bfloat16, kind="Internal")
    rs_in = [
        nc.dram_tensor(f"rs_in{i}", [cfg.TQ, D], dt.bfloat16, kind="Internal")
        for i in range(2)
    ]
    rs_out = [
        nc.dram_tensor(f"rs_out{i}", [cfg.TQC, D], dt.bfloat16, kind="Internal")
        for i in range(2)
    ]

    with tile.TileContext(nc) as tc, ExitStack() as ctx:
        # ---------------- constants ----------------
        cpool = ctx.enter_context(tc.tile_pool(name="const", bufs=1))
        agu_sb = cpool.tile([128, cfg.DCH, 4 * R], dt.bfloat16)
        nc.sync.dma_start(agu_sb[:], a_gu.rearrange("(c p) r -> p c r", p=128))
        bgu_sb = cpool.tile([4 * R, FS], dt.bfloat16)
        nc.sync.dma_start(bgu_sb[:], b_gu[:])
        adT_sb = cpool.tile([R, NFG, 128], dt.bfloat16)
        nc.sync.dma_start(adT_sb[:], a_dT.rearrange("r (c p) -> r c p", p=128))
        bd_sb = cpool.tile([R, D], dt.bfloat16)
        nc.sync.dma_start(bd_sb[:], b_d[:])
        xa_sb = cpool.tile([4 * R, T], dt.bfloat16)   # rows 0:16 g, 32:48 u

        dqp_cm = tc.tile_pool(name="dq", bufs=1)
        dqp = dqp_cm.__enter__()

        # ------------- dequant v5 -------------
        # terms at 4x tensor_scalar; DVE chains its own terms with t_t (2x).
        # gpsimd consumes ADJACENT term pairs immediately (independent
        # pair-adds, then its own small tree) so the pm ring never backs the
        # DVE up behind a serial cross-engine chain.
        pool_pairs = [(2, 3), (5, 6), (9, 10), (12, 13)][:cfg.pool_terms // 2]
        pool_ks = {k for pr in pool_pairs for k in pr}

        def emit_deq(nib, amp, c0, G, fw, f0, wview, lsb=None, out_dram=None):
            """Dequant rows [128*c0, 128*(c0+G)) x cols [f0, f0+fw) into bf16
            weights at wview ([128, G, fw] AP); optionally add lsb ([128, GW])
            and/or DMA to out_dram."""
            GW = G * fw
            X = dqp.tile([128, G, fw], dt.bfloat16, tag="x", bufs=2, name="x")
            nc.sync.dma_start(
                X[:], nib[128 * c0:128 * (c0 + G), f0:f0 + fw]
                .rearrange("(g p) f -> p g f", p=128))
            S = dqp.tile([128, G, fw], dt.bfloat16, tag="am", bufs=2, name="S")
            nc.sync.dma_start(
                S[:], amp[128 * c0:128 * (c0 + G), f0:f0 + fw]
                .rearrange("(g p) f -> p g f", p=128))
            Xf = X[:].rearrange("p g f -> p (g f)")
            Sf = S[:].rearrange("p g f -> p (g f)")

            accs = [dqp.tile([128, GW], dt.bfloat16, tag=t, bufs=1, name=t)
                    for t in ("aA", "aB")]
            cur = None
            pend = None
            qs = []
            for k in range(1, 16):
                if k in pool_ks:
                    t = dqp.tile([128, GW], dt.bfloat16, tag="pm", bufs=3,
                                 name="pm")
                    nc.vector.tensor_scalar(t[:], Xf, float(k), cdel[k],
                                            ALU.is_equal, ALU.mult)
                    if pend is None:
                        pend = t
                    else:
                        q = dqp.tile([128, GW], dt.bfloat16, tag="qs", bufs=3,
                                     name="qs")
                        nc.gpsimd.tensor_tensor(q[:], pend[:], t[:], ALU.add)
                        pend = None
                        qs.append(q)
                elif cur is None:
                    cur = accs[0]
                    nc.vector.tensor_scalar(cur[:], Xf, float(k), cdel[k],
                                            ALU.is_equal, ALU.mult)
                else:
                    t = dqp.tile([128, GW], dt.bfloat16, tag="tm", bufs=1,
                                 name="tm")
                    nc.vector.tensor_scalar(t[:], Xf, float(k), cdel[k],
                                            ALU.is_equal, ALU.mult)
                    nxt = accs[1] if cur is accs[0] else accs[0]
                    nc.vector.tensor_tensor(nxt[:], cur[:], t[:], ALU.add)
                    cur = nxt
            for q in qs:
                nxt = accs[1] if cur is accs[0] else accs[0]
                nc.vector.tensor_tensor(nxt[:], cur[:], q[:], ALU.add)
                cur = nxt
            # w = (acc + c0) * S  [+ lsb]
            wflat = wview.rearrange("p g f -> p (g f)")
            nxt = accs[1] if cur is accs[0] else accs[0]
            nc.vector.tensor_scalar(nxt[:], cur[:], c0v, None, ALU.add)
            if lsb is None:
                nc.vector.tensor_tensor(wflat[:], nxt[:], Sf, ALU.mult)
            else:
                nc.vector.tensor_tensor(nxt[:], nxt[:], Sf, ALU.mult)
                nc.vector.tensor_tensor(wflat[:], nxt[:], lsb[:], ALU.add)
            if out_dram is not None:
                nc.sync.dma_start(
                    out_dram[128 * c0:128 * (c0 + G), f0:f0 + fw]
                    .rearrange("(g p) f -> p g f", p=128),
                    wview)

        def emit_deq_act(nib, amp, c0, G, fw, f0, wview, lsb=None,
                         out_dram=None):
            """ACT+gpsimd dequant island (no DVE): per term k,
            Exp(-20*Square(X-k) + ln|cdel_k|) = |cdel_k| * 1[X==k] exactly on
            integer symbols; gpsimd chains add/subtract by sign(cdel_k); ACT
            adds c0 via Copy bias; gpsimd applies the absmax scale."""
            GW = G * fw
            X = dqp.tile([128, G, fw], dt.bfloat16, tag="ix", bufs=2,
                         name="ix")
            nc.sync.dma_start(
                X[:], nib[128 * c0:128 * (c0 + G), f0:f0 + fw]
                .rearrange("(g p) f -> p g f", p=128))
            S = dqp.tile([128, G, fw], dt.bfloat16, tag="iam", bufs=2,
                         name="iS")
            nc.sync.dma_start(
                S[:], amp[128 * c0:128 * (c0 + G), f0:f0 + fw]
                .rearrange("(g p) f -> p g f", p=128))
            Xf = X[:].rearrange("p g f -> p (g f)")
            Sf = S[:].rearrange("p g f -> p (g f)")

            live_ks = [k for k in range(1, 16) if abs(cdel[k]) > 1e-30]
            sgn = 1.0 if (not live_ks or cdel[live_ks[0]] > 0) else -1.0
            cur = None
            acc = None
            for k in live_ks:
                sq = dqp.tile([128, GW], dt.bfloat16, tag="isq", bufs=1,
                              name="isq")
                nc.scalar.activation(sq[:], Xf, AFT.Square, bias=-float(k))
                t = dqp.tile([128, GW], dt.bfloat16, tag="itm", bufs=2,
                             name="itm")
                nc.scalar.activation(t[:], sq[:], AFT.Exp,
                                     bias=float(np.log(abs(cdel[k]))),
                                     scale=-20.0)
                op = ALU.add if cdel[k] * sgn > 0 else ALU.subtract
                if cur is None:
                    cur = t                       # init: acc := first term
                elif acc is None:
                    acc = dqp.tile([128, GW], dt.bfloat16, tag="iac", bufs=1,
                                   name="iac")
                    nc.gpsimd.tensor_tensor(acc[:], cur[:], t[:], op)
                    cur = acc
                else:
                    nc.gpsimd.tensor_tensor(acc[:], acc[:], t[:], op)
            a2 = dqp.tile([128, GW], dt.bfloat16, tag="ia2", bufs=1,
                          name="ia2")
            wflat = wview.rearrange("p g f -> p (g f)")
            if cur is None:
                # degenerate: all cdel zero -> weights are c0 * S (+ lsb)
                nc.scalar.activation(a2[:], Sf, AFT.Copy, scale=c0v)
                if lsb is None:
                    nc.scalar.copy(wflat[:], a2[:])
                else:
                    nc.gpsimd.tensor_tensor(wflat[:], a2[:], lsb[:], ALU.add)
            else:
                # a2 = sgn * acc + c0
                nc.scalar.activation(a2[:], cur[:], AFT.Copy, bias=c0v,
                                     scale=sgn)
                if lsb is None:
                    nc.gpsimd.tensor_tensor(wflat[:], a2[:], Sf, ALU.mult)
                else:
                    nc.gpsimd.tensor_tensor(a2[:], a2[:], Sf, ALU.mult)
                    nc.gpsimd.tensor_tensor(wflat[:], a2[:], lsb[:], ALU.add)
            if out_dram is not None:
                nc.sync.dma_start(
                    out_dram[128 * c0:128 * (c0 + G), f0:f0 + fw]
                    .rearrange("(g p) f -> p g f", p=128),
                    wview)

        # ------------- phase 1 -------------
        wtiles = {}          # si -> (wg, wu); even/odd parity tags share bufs
        tasks = []

        def pop_tasks(n):
            for _ in range(n):
                if tasks:
                    tasks.pop(0)()

        fg0s = np.cumsum([0] + cfg.slices).tolist()
        nsl = len(cfg.slices)

        with tc.tile_pool(name="w", bufs=1) as wp, \
             tc.tile_pool(name="xt", bufs=1) as xtp, \
             tc.tile_pool(name="p1", bufs=1) as p1p, \
             tc.tile_pool(name="ps1", bufs=1, space="PSUM") as psp:

            def open_wpool(si):
                if si in wtiles:
                    return
                fw = cfg.slices[si] * 128
                par = "eo"[si % 2]
                wg = wp.tile([128, cfg.DCH, fw], dt.bfloat16, tag=f"wg{par}",
                             name=f"wg{si}")
                wu = wp.tile([128, cfg.DCH, fw], dt.bfloat16, tag=f"wu{par}",
                             name=f"wu{si}")
                wtiles[si] = (wg, wu)

            def make_gu_tasks(si):
                nfg = cfg.slices[si]
                fw = nfg * 128
                f0 = fg0s[si] * 128
                wg, wu = wtiles[si]
                out = []
                for (c0, G) in cfg.slice_groups(nfg):
                    out.append(lambda c0=c0, G=G: emit_deq(
                        g_nib, g_amp, c0, G, fw, f0, wg[:, c0:c0 + G, :]))
                    out.append(lambda c0=c0, G=G: emit_deq(
                        u_nib, u_amp, c0, G, fw, f0, wu[:, c0:c0 + G, :]))
                return out

            def make_down_tasks():
                """One task per (f-chunk c, d-quarter h): lora piece on PE,
                then dequant+fold -> wd_dram.  Most pieces go to the ACT+Pool
                island; the rest to the DVE path."""
                out = []
                pw = 1024
                npc = D // pw
                idx = 0
                for c in range(NFG):
                    for h in range(npc):
                        island = (idx % 4) < 3
                        def f(c=c, h=h, island=island):
                            ltag, wtag = (("ilb", "iwd") if island
                                          else ("lsb", "wdt"))
                            lsb = dqp.tile([128, pw], dt.bfloat16, tag=ltag,
                                           bufs=2, name=ltag)
                            for j in range(pw // 512):
                                lp = psp.tile([128, 512], dt.float32,
                                              tag="lp", bufs=2, name="lp")
                                dsl = slice(pw * h + 512 * j,
                                            pw * h + 512 * (j + 1))
                                nc.tensor.matmul(lp[:], adT_sb[:, c, :],
                                                 bd_sb[:, dsl],
                                                 start=True, stop=True)
                                nc.scalar.copy(lsb[:, 512 * j:512 * (j + 1)],
                                               lp[:])
                            wt = dqp.tile([128, 1, pw], dt.bfloat16, tag=wtag,
                                          bufs=1, name=wtag)
                            emit = emit_deq_act if island else emit_deq
                            emit(d_nib, d_amp, c, 1, pw, pw * h, wt[:],
                                 lsb=lsb, out_dram=wd_dram)
                        out.append(f)
                        idx += 1
                return out

            down_tasks = make_down_tasks()

            def take_down(n):
                batch, down_tasks[:n] = down_tasks[:n], []
                return batch

            n_xh = 2 if cfg.DCH >= 8 else 1
            CH = cfg.DCH // n_xh      # chunks per x-half

            open_wpool(0)
            s0_tasks = make_gu_tasks(0)
            if nsl > 1:
                open_wpool(1)
                tasks.extend(make_gu_tasks(1))
                tasks.extend(take_down(8))

            for si in range(nsl):
                fg0, nfg = fg0s[si], cfg.slices[si]
                fw = nfg * 128
                wg, wu = wtiles[si]
                quota = (len(tasks) + cfg.NT - 1) // cfg.NT if tasks else 0

                for t in range(cfg.NT):
                    tt = slice(TTW * t, TTW * (t + 1))
                    xth = []
                    for h in range(n_xh):
                        xh = xtp.tile([128, CH, TTW], dt.bfloat16, tag="xt",
                                      bufs=3, name="xh")
                        nc.sync.dma_start(
                            xh[:], xT[128 * CH * h:128 * CH * (h + 1), tt]
                            .rearrange("(c p) t -> p c t", p=128))
                        xth.append(xh)

                    if si == 0:
                        # fused gate+up lora projection: [64, TTW]
                        pxa = psp.tile([4 * R, TTW], dt.float32, tag="pxa",
                                       bufs=2, name="pxa")
                        for ci in range(cfg.DCH):
                            nc.tensor.matmul(
                                pxa[:], agu_sb[:, ci, :],
                                xth[ci // CH][:, ci % CH, :],
                                start=(ci == 0), stop=(ci == cfg.DCH - 1))
                        nc.scalar.copy(xa_sb[:, tt], pxa[:])

                    x3b = p1p.tile([128, nfg, TTW], dt.bfloat16, tag="x3b",
                                   bufs=2, name="x3b")
                    for fg in range(nfg):
                        fa = slice(128 * (fg0 + fg), 128 * (fg0 + fg + 1))
                        fl = slice(128 * fg, 128 * (fg + 1))
                        pg = psp.tile([128, TTW], dt.float32, tag="pg", bufs=2,
                                      name="pg")
                        pu = psp.tile([128, TTW], dt.float32, tag="pu", bufs=2,
                                      name="pu")
                        if si == 0 and t == 0 and s0_tasks:
                            # slice-0 first tile: dequant group -> matmul chunks
                            groups = cfg.slice_groups(nfg)
                            for gi, (c0, G) in enumerate(groups):
                                pop2 = s0_tasks[:2]
                                del s0_tasks[:2]
                                for fn in pop2:
                                    fn()
                                for ci in range(c0, c0 + G):
                                    nc.tensor.matmul(
                                        pg[:], wg[:, ci, fl],
                                        xth[ci // CH][:, ci % CH, :],
                                        start=(ci == 0), stop=False)
                                for ci in range(c0, c0 + G):
                                    nc.tensor.matmul(
                                        pu[:], wu[:, ci, fl],
                                        xth[ci // CH][:, ci % CH, :],
                                        start=(ci == 0), stop=False)
                        else:
                            for ci in range(cfg.DCH):
                                nc.tensor.matmul(pg[:], wg[:, ci, fl],
                                                 xth[ci // CH][:, ci % CH, :],
                                                 start=(ci == 0), stop=False)
                            for ci in range(cfg.DCH):
                                nc.tensor.matmul(pu[:], wu[:, ci, fl],
                                                 xth[ci // CH][:, ci % CH, :],
                                                 start=(ci == 0), stop=False)
                        nc.tensor.matmul(pg[:], bgu_sb[0:R, fa],
                                         xa_sb[0:R, tt], start=False, stop=True)
                        nc.tensor.matmul(pu[:], bgu_sb[2 * R:3 * R, fa],
                                         xa_sb[2 * R:3 * R, tt],
                                         start=False, stop=True)
                        sg = p1p.tile([128, TTW], dt.bfloat16, tag="sg",
                                      bufs=2, name="sg")
                        pub = p1p.tile([128, TTW], dt.bfloat16, tag="pub",
                                       bufs=2, name="pub")
                        nc.scalar.copy(pub[:], pu[:])
                        if cfg.use_silu:
                            nc.scalar.activation(sg[:], pg[:], AFT.Silu)
                            nc.gpsimd.tensor_tensor(x3b[:, fg, :], sg[:],
                                                    pub[:], ALU.mult)
                        else:
                            nc.scalar.activation(sg[:], pg[:], AFT.Sigmoid)
                            pgb = p1p.tile([128, TTW], dt.bfloat16, tag="pgb",
                                           bufs=2, name="pgb")
                            nc.scalar.copy(pgb[:], pg[:])
                            nc.gpsimd.tensor_tensor(sg[:], sg[:], pgb[:],
                                                    ALU.mult)
                            nc.gpsimd.tensor_tensor(x3b[:, fg, :], sg[:],
                                                    pub[:], ALU.mult)
                    nc.sync.dma_start(
                        x3_dram[128 * fg0:128 * fg0 + fw, tt]
                        .rearrange("(g p) t -> p g t", p=128),
                        x3b[:])
                    pop_tasks(quota)

                # queue what dequants next
                if si + 2 < nsl:
                    open_wpool(si + 2)
                    tasks.extend(make_gu_tasks(si + 2))
                    tasks.extend(take_down(8))
                else:
                    tasks.extend(take_down(len(down_tasks)))

            pop_tasks(len(tasks))       # down-weight dequant tail

        dqp_cm.__exit__(None, None, None)

        # ------------- phase 2 -------------
        with tc.tile_pool(name="p2", bufs=1) as p2p, \
             tc.tile_pool(name="wd", bufs=1) as wdp, \
             tc.tile_pool(name="ps2", bufs=1, space="PSUM") as ps2:
            wd_sb = wdp.tile([128, NFG, D], dt.bfloat16, name="wd_sb")
            for c in range(NFG):
                nc.sync.dma_start(wd_sb[:, c, :],
                                  wd_dram[128 * c:128 * (c + 1), :])
            NTGQ = cfg.TQ // 128
            for q in range(cfg.n_q):
                j = q % 2
                for tgl in range(NTGQ):
                    tg = NTGQ * q + tgl
                    tsl = slice(128 * tg, 128 * (tg + 1))
                    x3g = p2p.tile([128, NFG, 128], dt.bfloat16, tag="x3g",
                                   bufs=3, name="x3g")
                    nc.sync.dma_start(
                        x3g[:], x3_dram[:, tsl]
                        .rearrange("(c p) t -> p c t", p=128))
                    yb = p2p.tile([128, D], dt.bfloat16, tag="yb", bufs=2,
                                  name="yb")
                    for dh2 in range(2):
                        pds = [ps2.tile([128, 512], dt.float32, tag=f"pd{dj}",
                                        bufs=2, name=f"pd{dj}")
                               for dj in range(4)]
                        for ci in range(NFG):
                            for dj in range(4):
                                dsl = slice(2048 * dh2 + 512 * dj,
                                            2048 * dh2 + 512 * (dj + 1))
                                nc.tensor.matmul(
                                    pds[dj][:], x3g[:, ci, :],
                                    wd_sb[:, ci, dsl],
                                    start=(ci == 0), stop=(ci == NFG - 1))
                        for dj in range(4):
                            nc.scalar.copy(
                                yb[:, 2048 * dh2 + 512 * dj:
                                   2048 * dh2 + 512 * (dj + 1)],
                                pds[dj][:])
                    nc.sync.dma_start(
                        rs_in[j][128 * tgl:128 * (tgl + 1), :], yb[:])
                nc.gpsimd.collective_compute(
                    "ReduceScatter", ALU.add, replica_groups=rg,
                    ins=[rs_in[j][:, :].opt()],
                    outs=[rs_out[j][:, :].opt()],
                )
                rt = p2p.tile([128, D], dt.bfloat16, tag="rt", bufs=2,
                              name="rt")
                nc.sync.dma_start(rt[0:cfg.TQC, :], rs_out[j][:])
                yf = p2p.tile([128, D], dt.float32, tag="yf", bufs=2,
                              name="yf")
                nc.vector.tensor_scalar(yf[0:cfg.TQC, :], rt[0:cfg.TQC, :],
                                        1.0, None, ALU.mult)
                nc.sync.dma_start(
                    y_out[cfg.TQC * q:cfg.TQC * (q + 1), :],
                    yf[0:cfg.TQC, :])

    nc.compile()
    return nc


# ----------------- host side -----------------

_CACHE = {}


def _get_graph(cfg: Cfg, code):
    key = (cfg.D, cfg.T, cfg.F, cfg.ncores, cfg.use_silu, tuple(code))
    if key not in _CACHE:
        _CACHE[key] = build_graph(cfg, code)
    return _CACHE[key]


def _prep_inputs(cfg: Cfg, inputs):
    """Shard + lay out the full inputs for each core (marshalling only:
    transpose, nibble unpack, dtype casts, padding)."""
    D, T, F, FP, FS, R = cfg.D, cfg.T, cfg.F, cfg.FP, cfg.FS, cfg.R
    blk = cfg.block

    x = np.asarray(inputs["x"])
    xT = np.ascontiguousarray(x.T).astype(BF16)

    def nib_split(packed, rows, cols):
        """packed int32 words (one byte each) -> u8 nibble values [rows, cols]."""
        b = (np.asarray(packed).astype(np.int64) & 0xFF).astype(np.uint8)
        b = b.reshape(rows, cols // 2)
        out = np.empty((rows, cols), np.uint8)
        out[:, 0::2] = b >> 4
        out[:, 1::2] = b & 0xF
        return out

    # gate/up: [F, D] -> pad rows to FP -> transpose -> [D, FP]; shard cols
    def prep_gu(packed, absmax):
        nib = nib_split(packed, F, D)
        nib = np.concatenate([nib, np.zeros((FP - F, D), np.uint8)], 0)
        nibT = np.ascontiguousarray(nib.T).astype(BF16)  # [D, FP]
        am = np.asarray(absmax, np.float32).reshape(F, D // blk)
        am = np.concatenate([am, np.zeros((FP - F, D // blk), np.float32)], 0)
        amT = np.ascontiguousarray(am.T).astype(BF16)   # [D/blk, FP]
        return nibT, amT

    g_nibT, g_amT = prep_gu(inputs["w_gate_packed"], inputs["w_gate_absmax"])
    u_nibT, u_amT = prep_gu(inputs["w_up_packed"], inputs["w_up_absmax"])

    # down: [D, F] -> pad cols to FP -> transpose -> [FP, D]; shard rows
    d_nib = nib_split(inputs["w_down_packed"], D, F)
    d_nib = np.concatenate([d_nib, np.zeros((D, FP - F), np.uint8)], 1)
    d_nibT = np.ascontiguousarray(d_nib.T).astype(BF16)  # [FP, D]
    d_am = np.asarray(inputs["w_down_absmax"], np.float32).reshape(D, F // blk)
    d_am = np.concatenate([d_am, np.zeros((D, (FP - F) // blk), np.float32)], 1)
    d_amT = np.ascontiguousarray(d_am.T).astype(BF16)   # [FP/blk, D]

    # gate/up lora packing: a_gu [D, 64], b_gu [64, FP]
    a_gu = np.zeros((D, 4 * R), np.float32)
    a_gu[:, 0:R] = np.asarray(inputs["w_gate_lora_a"])
    a_gu[:, 2 * R:3 * R] = np.asarray(inputs["w_up_lora_a"])
    b_gu = np.zeros((4 * R, FP), np.float32)
    b_gu[0:R, :F] = np.asarray(inputs["w_gate_lora_b"])
    b_gu[2 * R:3 * R, :F] = np.asarray(inputs["w_up_lora_b"])

    a_dT = np.zeros((R, FP), np.float32)
    a_dT[:, :F] = np.asarray(inputs["w_down_lora_a"]).T
    b_d = np.asarray(inputs["w_down_lora_b"]).astype(BF16)

    code = np.asarray(inputs["code"], np.float32)

    g_amP = np.repeat(g_amT, blk, axis=0)     # [D, FP]
    u_amP = np.repeat(u_amT, blk, axis=0)
    d_amP = np.repeat(d_amT, blk, axis=0)     # [FP, D]

    in_maps = []
    for i in range(cfg.ncores):
        fsl = slice(FS * i, FS * (i + 1))
        in_maps.append({
            "xT": xT,
            "g_nib": np.ascontiguousarray(g_nibT[:, fsl]),
            "u_nib": np.ascontiguousarray(u_nibT[:, fsl]),
            "d_nib": np.ascontiguousarray(d_nibT[fsl]),
            "g_amp": np.ascontiguousarray(g_amP[:, fsl]),
            "u_amp": np.ascontiguousarray(u_amP[:, fsl]),
            "d_amp": np.ascontiguousarray(d_amP[fsl]),
            "a_gu": a_gu.astype(BF16),
            "b_gu": np.ascontiguousarray(b_gu[:, fsl]).astype(BF16),
            "a_dT": np.ascontiguousarray(a_dT[:, fsl]).astype(BF16),
            "b_d": b_d,
        })
    return in_maps, code


def _gather(cfg: Cfg, results):
    """Reassemble full [T, D] output from per-core quarter-row blocks."""
    y = np.empty((cfg.T, cfg.D), np.float32)
    for i in range(cfg.ncores):
        yi = results[i]["y_out"]
        for q in range(cfg.n_q):
            r0 = cfg.TQ * q + cfg.TQC * i
            y[r0:r0 + cfg.TQC] = yi[cfg.TQC * q:cfg.TQC * (q + 1)]
    return y


def run(cfg: Cfg, inputs, trace=False, **kwargs):
    in_maps, code = _prep_inputs(cfg, inputs)
    nc = _get_graph(cfg, [float(v) for v in code])
    res = run_bass_kernel_spmd(
        nc, in_maps, core_ids=list(range(cfg.ncores)), trace=trace, **kwargs
    )
    y = _gather(cfg, res.results)
    return y, res


def kernel(**inputs) -> np.ndarray:
    cfg = Cfg()
    y, _ = run(cfg, inputs)
    return y.astype(np.float32)
